# revision 7
# baseline (speedup 1.0000x reference)
"""Bass/TRN2 kernel for nn_DeepGeoConvSNN — 8-core data-parallel over batch.

Pipeline per core (16 of 128 batch elements):
  u/v channel-mix matmuls (PE) -> cross-core sync BatchNorm over batch ->
  LIF scan layer1 (DVE, sequential 480 steps, decay tensor) ->
  conv1d k=15 + 1x1 shortcut (PE, taps as accumulated matmuls, 2 taps/MM) ->
  sync BN -> LIF layer2 (DVE, 8 time-chunks in parallel + 32-step warmup;
  exact because decay=0.5 contracts to below f32 ulp) ->
  conv1d k=7 + 1x1 (PE) -> sync BN (per conv half, overlapped) ->
  LIF layer3 (2 sequential time-passes of 4 chunks + 28-step warmup) ->
  avgpool32 (DVE segmented reduce) -> sync BN -> folded FC (PE) -> out.

LIF scans use a negated-state formulation n = -u so each step is stock ops:
  spike s = (n < -vth) (tensor_scalar, written straight into the padded
  spike buffer), -v = s*vth + n (scalar_tensor_tensor), and the update
  n' = (-v)*d - i (tensor_tensor mult+sub, or one fused stt when d=0.5).

decay1 = exp(-1/(35*exp(-(0.8*curv+0.4*tang)))) is computed on host in
float32-faithful arithmetic: the ACT table Exp has ~1.1e-5 relative error,
which measurably flips spikes in this chaotic net (0.14 absmax output error
vs the 0.004 fp32-reordering envelope). All BatchNorm rstd values are
computed on device as reciprocal(sqrt(var+eps)) with one Newton refinement
(ACT Sqrt alone is ~7e-6, same problem; DVE reciprocal is ~6e-8).
"""
import sys
sys.path.insert(0, '/opt/trn_rl_repo')
import numpy as np

import concourse.bass as bass
from concourse import mybir, tile
from concourse.bass_utils import run_bass_kernel_spmd

F32 = mybir.dt.float32
AL = mybir.AluOpType
AF = mybir.ActivationFunctionType
AX = mybir.AxisListType

NCORES = 8
B, C, T = 128, 64, 480
Bs = B // NCORES            # 16 batch per core
C1, C2 = 128, 256
VTH1, VTH2 = 0.15, 0.3
L2_NC, L2_INT, L2_WU = 8, 60, 32
L2_S = 1 + L2_WU + L2_INT   # 93 state slots (slot 0 = zero init)
L2_I = L2_WU + L2_INT       # 92 injection slots
L3_NC, L3_INT, L3_WU = 4, 64, 28
L3_S = 1 + L3_WU + L3_INT   # 93
L3_I = L3_WU + L3_INT       # 92
P1T, P2T = 256, 224

MAXW_SYNC = 1  # walrus build here rejects >1 sync wait per instruction


def _split_waits(nc):
    n = 0
    for fn in nc.m.functions:
        for bb in fn.blocks:
            insts = bb.instructions
            out = []
            changed = False
            for inst in insts:
                si = inst.sync_info
                if si is not None and len(si.on_wait) > MAXW_SYNC:
                    w = list(si.on_wait)
                    excess, keep = w[:-MAXW_SYNC], w[-MAXW_SYNC:]
                    for k, sw in enumerate(excess):
                        out.append(mybir.InstNoOp(
                            name=f"{inst.name}-wsplit{k}", engine=inst.engine,
                            sync_info=mybir.SyncInfo(on_wait=[sw], on_update=[]),
                            bass_nofuse=True))
                        n += 1
                    si.on_wait = keep
                    changed = True
                out.append(inst)
            if changed:
                bb.instructions = out
    return n


def _rsqrt_refined(nc, pool, xe, pdim, fdim, tag):
    """rstd = 1/sqrt(xe) with one Newton step. xe: AP holding var+eps."""
    s0 = pool.tile([pdim, fdim], F32, tag=f"{tag}_s0")
    nc.scalar.activation(s0[:], xe, AF.Sqrt)
    r0 = pool.tile([pdim, fdim], F32, tag=f"{tag}_r0")
    nc.vector.reciprocal(r0[:], s0[:])
    t1 = pool.tile([pdim, fdim], F32, tag=f"{tag}_t1")
    nc.vector.tensor_tensor(t1[:], r0[:], r0[:], AL.mult)
    nc.vector.tensor_tensor(t1[:], t1[:], xe, AL.mult)
    nc.vector.tensor_scalar(t1[:], t1[:], -0.5, 1.5, AL.mult, AL.add)
    r1 = pool.tile([pdim, fdim], F32, tag=f"{tag}_r1")
    nc.vector.tensor_tensor(r1[:], r0[:], t1[:], AL.mult)
    return r1


def build(debug=False, repeat=1):
    nc = bass.Bass(num_devices=NCORES)
    ext = {}
    ext["u_in"] = nc.declare_dram_parameter("u", [C, Bs * T], F32, isOutput=False)
    ext["v_in"] = nc.declare_dram_parameter("v", [C, Bs * T], F32, isOutput=False)
    ext["dec_in"] = nc.declare_dram_parameter("dec", [C, Bs * T], F32, isOutput=False)
    ext["a_w"] = nc.declare_dram_parameter("a_w", [C, C], F32, isOutput=False)
    ext["wu_w"] = nc.declare_dram_parameter("wu_w", [C, C], F32, isOutput=False)
    ext["wv_w"] = nc.declare_dram_parameter("wv_w", [C, C], F32, isOutput=False)
    ext["ginj_in"] = nc.declare_dram_parameter("ginj", [C, 1], F32, isOutput=False)
    ext["binj_in"] = nc.declare_dram_parameter("binj", [C, 1], F32, isOutput=False)
    ext["w1p_in"] = nc.declare_dram_parameter("w1p", [C1, 8 * C1], F32, isOutput=False)
    ext["sc1_in"] = nc.declare_dram_parameter("sc1w", [C, C1], F32, isOutput=False)
    ext["g1_in"] = nc.declare_dram_parameter("g1", [C1, 1], F32, isOutput=False)
    ext["b1s_in"] = nc.declare_dram_parameter("b1s", [C1, 1], F32, isOutput=False)
    ext["w2_in"] = nc.declare_dram_parameter("w2", [C1, 14 * C1], F32, isOutput=False)
    ext["sc2_in"] = nc.declare_dram_parameter("sc2w", [C1, 2 * C1], F32, isOutput=False)
    ext["g2_in"] = nc.declare_dram_parameter("g2", [C1, 2], F32, isOutput=False)
    ext["b2s_in"] = nc.declare_dram_parameter("b2s", [C1, 2], F32, isOutput=False)
    ext["gfc_in"] = nc.declare_dram_parameter("gfc", [C1, 30 * 4], F32, isOutput=False)
    ext["hfc_in"] = nc.declare_dram_parameter("hfc", [4, 1], F32, isOutput=False)
    ext["o_out"] = nc.declare_dram_parameter("o", [4, Bs], F32, isOutput=True)
    if debug:
        ext["dbg"] = {
            "dbg_pre": nc.declare_dram_parameter("dbg_pre", [C, Bs * T], F32, isOutput=True),
            "dbg_iinj": nc.declare_dram_parameter("dbg_iinj", [C, Bs * T], F32, isOutput=True),
            "dbg_sp1": nc.declare_dram_parameter("dbg_sp1", [C, Bs * T], F32, isOutput=True),
            "dbg_inj2": nc.declare_dram_parameter("dbg_inj2", [C1, Bs * T], F32, isOutput=True),
            "dbg_sp2": nc.declare_dram_parameter("dbg_sp2", [C1, Bs * T], F32, isOutput=True),
            "dbg_sp3": nc.declare_dram_parameter("dbg_sp3", [C1, 2 * 2 * Bs * L3_NC * L3_INT], F32, isOutput=True),
            "dbg_x": nc.declare_dram_parameter("dbg_x", [C1, 30 * 17], F32, isOutput=True),
        }

    with tile.TileContext(nc, pool_alloc_mode="queue") as tc:
        for rep in range(repeat):
            _emit_body(nc, tc, ext, debug and rep == 0)

    _split_waits(nc)
    return nc


def _emit_body(nc, tc, ext, debug):
    dbg = ext.get("dbg") if debug else None
    core_ids = list(range(NCORES))
    o_out = ext["o_out"]

    P = lambda name, side: tc.alloc_tile_pool(name=name, bufs=1, side=side)

    p0 = P("p0", "left")
    pdram = tc.alloc_tile_pool(name="pdram", bufs=1, space="DRAM")

    wa = p0.tile([C, C], F32, tag="wa")
    wwu = p0.tile([C, C], F32, tag="wwu")
    wwv = p0.tile([C, C], F32, tag="wwv")
    ginj = p0.tile([C, 1], F32, tag="ginj")
    binj = p0.tile([C, 1], F32, tag="binj")
    w1p = p0.tile([C1, 8 * C1], F32, tag="w1p")
    sc1w = p0.tile([C, C1], F32, tag="sc1w")
    g1 = p0.tile([C1, 1], F32, tag="g1")
    b1s = p0.tile([C1, 1], F32, tag="b1s")
    w2 = p0.tile([C1, 14 * C1], F32, tag="w2")
    sc2w = p0.tile([C1, 2 * C1], F32, tag="sc2w")
    g2 = p0.tile([C1, 2], F32, tag="g2")
    b2s = p0.tile([C1, 2], F32, tag="b2s")
    gfc = p0.tile([C1, 30 * 4], F32, tag="gfc")
    hfc = p0.tile([4, 1], F32, tag="hfc")
    for t_, s_ in [(wa, ext["a_w"]), (wwu, ext["wu_w"]), (wwv, ext["wv_w"]),
                   (ginj, ext["ginj_in"]), (binj, ext["binj_in"]),
                   (w1p, ext["w1p_in"]), (sc1w, ext["sc1_in"]), (g1, ext["g1_in"]),
                   (b1s, ext["b1s_in"]), (w2, ext["w2_in"]), (sc2w, ext["sc2_in"]),
                   (g2, ext["g2_in"]), (b2s, ext["b2s_in"]), (gfc, ext["gfc_in"]),
                   (hfc, ext["hfc_in"])]:
        nc.sync.dma_start(t_[:], s_[:])

    X = p0.tile([C1, 30 * 17], F32, tag="X")
    stats = p0.tile([C1, 64], F32, tag="stats")

    # ============ phase A ============
    p_uv = P("p_uv", "right")
    u_sb = p_uv.tile([C, Bs * T], F32, tag="u_sb")
    v_sb = p_uv.tile([C, Bs * T], F32, tag="v_sb")
    nc.sync.dma_start(u_sb[:], ext["u_in"][:])
    nc.sync.dma_start(v_sb[:], ext["v_in"][:])

    p_pre = P("p_pre", "left")
    pre_sb = p_pre.tile([C, Bs * T], F32, tag="pre_sb")
    stats1 = p_pre.tile([C, 2 * T], F32, tag="stats1")

    psA = tc.alloc_tile_pool(name="psA", bufs=1, space="PSUM")
    scrA = P("scrA", "right")
    presum_ps = psA.tile([C, T], F32, tag="presum")
    for b in range(Bs):
        sl = slice(b * T, (b + 1) * T)
        ua_ps = psA.tile([C, T], F32, tag="av_ps", bufs=4)
        va_ps = psA.tile([C, T], F32, tag="av_ps", bufs=4)
        nc.tensor.matmul(ua_ps[:], wa[:], u_sb[:, sl], start=True, stop=True)
        nc.tensor.matmul(va_ps[:], wa[:], v_sb[:, sl], start=True, stop=True)
        ua_sb = scrA.tile([C, T], F32, tag="av_sb", bufs=4)
        va_sb = scrA.tile([C, T], F32, tag="av_sb", bufs=4)
        nc.scalar.activation(ua_sb[:], ua_ps[:], AF.Copy)
        nc.scalar.activation(va_sb[:], va_ps[:], AF.Copy)
        pre_ps = psA.tile([C, T], F32, tag="pre_ps", bufs=2)
        nc.tensor.matmul(pre_ps[:], wwu[:], ua_sb[:], start=True, stop=False)
        nc.tensor.matmul(pre_ps[:], wwv[:], va_sb[:], start=False, stop=True)
        nc.tensor.matmul(presum_ps[:], wwu[:], ua_sb[:], start=(b == 0), stop=False)
        nc.tensor.matmul(presum_ps[:], wwv[:], va_sb[:], start=False, stop=(b == Bs - 1))
        nc.scalar.activation(pre_sb[:, sl], pre_ps[:], AF.Copy)
        if b == 0:
            nc.scalar.activation(stats1[:, T:2 * T], pre_ps[:], AF.Square)
        else:
            sq = scrA.tile([C, T], F32, tag="sq_sb", bufs=2)
            nc.scalar.activation(sq[:], pre_ps[:], AF.Square)
            nc.vector.tensor_tensor(stats1[:, T:2 * T], stats1[:, T:2 * T], sq[:], AL.add)
    nc.scalar.activation(stats1[:, 0:T], presum_ps[:], AF.Copy)
    scrA.release()
    psA.release()
    p_uv.release()

    # ---- allreduce 1: per-(c,t) sum & sumsq over batch ----
    ar1_i = pdram.tile([C, 2 * T], F32, tag="ar1_i")
    ar1_o = pdram.tile([C, 2 * T], F32, tag="ar1_o", addr_space="Shared")
    nc.sync.dma_start(ar1_i[:], stats1[:])
    nc.gpsimd.collective_compute("AllReduce", AL.add, replica_groups=[core_ids],
                                 ins=[ar1_i.opt()], outs=[ar1_o.opt()])
    nc.sync.dma_start(stats1[:], ar1_o[:])

    # ---- i_inj = pre*R - Q ----
    p_l1 = P("p_l1", "right")
    I1 = p_l1.tile([C, Bs * T], F32, tag="I1")
    N1 = p_l1.tile([C, Bs * 2], F32, tag="N1")     # rotating 2-slot state
    dec_sb = p_l1.tile([C, Bs * T], F32, tag="dec_sb")
    nc.sync.dma_start(dec_sb[:], ext["dec_in"][:])

    scrB = P("scrB", "left")
    m_t = scrB.tile([C, T], F32, tag="m_t")
    nc.vector.tensor_scalar(m_t[:], stats1[:, 0:T], 1.0 / 128.0, None, AL.mult)
    esq = scrB.tile([C, T], F32, tag="esq")
    nc.vector.tensor_scalar(esq[:], stats1[:, T:2 * T], 1.0 / 128.0, None, AL.mult)
    msq = scrB.tile([C, T], F32, tag="msq")
    nc.vector.tensor_tensor(msq[:], m_t[:], m_t[:], AL.mult)
    xe = scrB.tile([C, T], F32, tag="xe")
    nc.vector.tensor_tensor(xe[:], esq[:], msq[:], AL.subtract)
    nc.vector.tensor_scalar(xe[:], xe[:], 1e-5, None, AL.add)
    rstd = _rsqrt_refined(nc, scrB, xe[:], C, T, "bninj")
    R_t = scrB.tile([C, T], F32, tag="R_t")
    nc.vector.tensor_scalar(R_t[:], rstd[:], ginj[:], None, AL.mult)
    Q_t = scrB.tile([C, T], F32, tag="Q_t")
    nc.vector.tensor_tensor(Q_t[:], m_t[:], R_t[:], AL.mult)
    nc.vector.tensor_scalar(Q_t[:], Q_t[:], binj[:], None, AL.subtract)
    for b in range(Bs):
        sl = slice(b * T, (b + 1) * T)
        tb = scrB.tile([C, T], F32, tag="tb", bufs=2)
        nc.vector.tensor_tensor(tb[:], pre_sb[:, sl], R_t[:], AL.mult)
        nc.vector.tensor_tensor(I1[:, sl], tb[:], Q_t[:], AL.subtract)
    if debug:
        nc.sync.dma_start(dbg["dbg_pre"][:], pre_sb[:])
        nc.sync.dma_start(dbg["dbg_iinj"][:], I1[:])
    scrB.release()
    p_pre.release()

    # ============ phase D: layer-1 LIF scan ============
    SPW = 496  # 7 zero | 480 spikes | 9 zero (B half shifted +1)
    p_sp1 = P("p_sp1", "left")
    sppad = p_sp1.tile([C1, Bs * SPW], F32, tag="sppad")
    nc.gpsimd.memset(sppad[:], 0.0)

    n1v = N1[:].rearrange("p (b s) -> p b s", b=Bs)
    i1v = I1[:].rearrange("p (b s) -> p b s", b=Bs)
    dv = dec_sb[:].rearrange("p (b s) -> p b s", b=Bs)
    spav = sppad[0:C, :].rearrange("p (b s) -> p b s", b=Bs)

    scrD = P("scrD", "right")
    for t in range(T):
        cur = t % 2          # slot holding N[t]
        nxt = (t + 1) % 2    # slot for N[t+1]
        if t == 0:
            nc.vector.tensor_scalar(n1v[:, :, 1], i1v[:, :, 0], -1.0, None, AL.mult)
            continue
        sp_col = spav[:, :, 7 + (t - 1)]
        nc.vector.tensor_scalar(sp_col, n1v[:, :, cur], -VTH1, None, AL.is_lt)
        nv = scrD.tile([C, Bs], F32, tag="nv", bufs=2)
        nc.vector.scalar_tensor_tensor(nv[:], sp_col, VTH1, n1v[:, :, cur], AL.mult, AL.add)
        nw = scrD.tile([C, Bs], F32, tag="nw", bufs=2)
        nc.vector.tensor_tensor(nw[:], nv[:], dv[:, :, t], AL.mult)
        nc.vector.tensor_tensor(n1v[:, :, nxt], nw[:], i1v[:, :, t], AL.subtract)
    nc.vector.tensor_scalar(spav[:, :, 7 + T - 1], n1v[:, :, T % 2], -VTH1, None, AL.is_lt)
    scrD.release()

    if debug:
        dsp1 = dbg["dbg_sp1"][:].rearrange("p (b s) -> p b s", b=Bs)
        nc.sync.dma_start(dsp1[:, :, :], spav[:, :, 7:7 + T])
    p_l1.release()

    # shifted copy for tap pairing: SPB[64+c, b, j] = SPA[c, b, j+1]
    spbv = sppad[C:C1, :].rearrange("p (b s) -> p b s", b=Bs)
    for b in range(Bs):
        nc.sync.dma_start(spbv[:, b, 0:SPW - 1], spav[:, b, 1:SPW])

    # ============ phase E: conv1 + shortcut + BN1 stats ============
    p_c1 = P("p_c1", "right")
    c1_sb = p_c1.tile([C1, Bs * T], F32, tag="c1_sb")
    s1_sb = p_c1.tile([C1, Bs * T], F32, tag="s1_sb")
    c1sum = p_c1.tile([C1, Bs], F32, tag="c1sum")
    c1sq = p_c1.tile([C1, Bs], F32, tag="c1sq")
    sppv = sppad[:].rearrange("p (b s) -> p b s", b=Bs)

    psE = tc.alloc_tile_pool(name="psE", bufs=1, space="PSUM")
    scrE = P("scrE", "right")
    for b in range(Bs):
        sl = slice(b * T, (b + 1) * T)
        c1_ps = psE.tile([C1, T], F32, tag="c1_ps", bufs=2)
        for j in range(8):
            nc.tensor.matmul(c1_ps[:], w1p[:, j * C1:(j + 1) * C1],
                             sppv[:, b, 2 * j:2 * j + T],
                             start=(j == 0), stop=(j == 7))
        s1_ps = psE.tile([C1, T], F32, tag="s1_ps", bufs=2)
        nc.tensor.matmul(s1_ps[:], sc1w[:], spav[:, b, 7:7 + T], start=True, stop=True)
        nc.scalar.activation(c1_sb[:, sl], c1_ps[:], AF.Copy, accum_out=c1sum[:, b:b + 1])
        sqe = scrE.tile([C1, T], F32, tag="sqe", bufs=2)
        nc.scalar.activation(sqe[:], c1_ps[:], AF.Square, accum_out=c1sq[:, b:b + 1])
        nc.scalar.activation(s1_sb[:, sl], s1_ps[:], AF.Copy)
    scrE.release()
    psE.release()
    p_sp1.release()

    # ---- allreduce 2: BN1 ----
    nc.vector.tensor_reduce(stats[:, 0:1], c1sum[:], axis=AX.X, op=AL.add)
    nc.vector.tensor_reduce(stats[:, 1:2], c1sq[:], axis=AX.X, op=AL.add)
    ar2_i = pdram.tile([C1, 2], F32, tag="ar2_i")
    ar2_o = pdram.tile([C1, 2], F32, tag="ar2_o", addr_space="Shared")
    nc.sync.dma_start(ar2_i[:], stats[:, 0:2])
    nc.gpsimd.collective_compute("AllReduce", AL.add, replica_groups=[core_ids],
                                 ins=[ar2_i.opt()], outs=[ar2_o.opt()])
    nc.sync.dma_start(stats[:, 2:4], ar2_o[:])

    NBT = float(B * T)
    nc.vector.tensor_scalar(stats[:, 4:5], stats[:, 2:3], 1.0 / NBT, None, AL.mult)
    nc.vector.tensor_scalar(stats[:, 5:6], stats[:, 3:4], 1.0 / NBT, None, AL.mult)
    nc.vector.tensor_tensor(stats[:, 6:7], stats[:, 4:5], stats[:, 4:5], AL.mult)
    nc.vector.tensor_tensor(stats[:, 6:7], stats[:, 5:6], stats[:, 6:7], AL.subtract)
    nc.vector.tensor_scalar(stats[:, 6:7], stats[:, 6:7], 1e-5, None, AL.add)
    scrF = P("scrF", "left")
    rstd1 = _rsqrt_refined(nc, scrF, stats[:, 6:7], C1, 1, "bn1")
    nc.vector.tensor_scalar(stats[:, 7:8], rstd1[:], g1[:], None, AL.mult)
    nc.vector.tensor_scalar(stats[:, 8:9], stats[:, 7:8], -1.0, None, AL.mult)
    nc.vector.scalar_tensor_tensor(stats[:, 9:10], stats[:, 8:9], stats[:, 4:5],
                                   b1s[:], AL.mult, AL.add)
    scrF.release()

    # ============ phase G: build I2c, layer-2 scan ============
    p_l2 = P("p_l2", "left")
    I2c = p_l2.tile([C1, Bs * L2_NC * L2_I], F32, tag="I2c")
    N2 = p_l2.tile([C1, Bs * L2_NC * 2], F32, tag="N2")
    i2v = I2c[:].rearrange("p (b c s) -> p b c s", b=Bs, c=L2_NC)
    n2v = N2[:].rearrange("p (b c s) -> p b c s", b=Bs, c=L2_NC)
    nc.gpsimd.memset(n2v[:, :, :, 0], 0.0)
    nc.gpsimd.memset(i2v[:, :, 0, 0:L2_WU], 0.0)

    c1bv = c1_sb[:].rearrange("p (b s) -> p b s", b=Bs)
    s1bv = s1_sb[:].rearrange("p (b s) -> p b s", b=Bs)
    scrG = P("scrG", "right")
    for c in range(L2_NC):
        tmp = scrG.tile([C1, Bs * L2_INT], F32, tag="tmpg", bufs=2)
        tv = tmp[:].rearrange("p (b s) -> p b s", b=Bs)
        nc.vector.tensor_scalar(tv[:, :, :], c1bv[:, :, c * L2_INT:(c + 1) * L2_INT],
                                stats[:, 7:8], stats[:, 9:10], AL.mult, AL.add)
        nc.vector.tensor_tensor(i2v[:, :, c, L2_WU:L2_I], tv[:, :, :],
                                s1bv[:, :, c * L2_INT:(c + 1) * L2_INT], AL.add)
    for c in range(1, L2_NC):
        nc.vector.tensor_scalar(i2v[:, :, c, 0:L2_WU],
                                i2v[:, :, c - 1, L2_I - L2_WU:L2_I], 1.0, None, AL.mult)
    scrG.release()
    if debug:
        di2 = dbg["dbg_inj2"][:].rearrange("p (b c s) -> p b c s", b=Bs, c=L2_NC, s=L2_INT)
        nc.sync.dma_start(di2[:, :, :, :], i2v[:, :, :, L2_WU:L2_I])
    p_c1.release()

    p_sp2 = P("p_sp2", "right")
    SP2W = 486
    sp2a = p_sp2.tile([C1, Bs * SP2W], F32, tag="sp2a")
    nc.gpsimd.memset(sp2a[:], 0.0)
    sp2v = sp2a[:].rearrange("p (b s) -> p b s", b=Bs)

    scrH = P("scrH", "right")
    for s in range(1, L2_S):
        cur, nxt = (s - 1) % 2, s % 2
        if s - 1 >= 1 + L2_WU:
            off = 3 + (s - 1 - (1 + L2_WU))
            sp_loc = sp2v[:, :, off:off + (L2_NC - 1) * L2_INT + 1:L2_INT]
        else:
            spscr = scrH.tile([C1, Bs * L2_NC], F32, tag="spscr", bufs=2)
            sp_loc = spscr[:].rearrange("p (b c) -> p b c", b=Bs)
        nc.vector.tensor_scalar(sp_loc, n2v[:, :, :, cur], -VTH2, None, AL.is_lt)
        nv = scrH.tile([C1, Bs * L2_NC], F32, tag="nv2", bufs=2)
        nvv = nv[:].rearrange("p (b c) -> p b c", b=Bs)
        nc.vector.scalar_tensor_tensor(nvv, sp_loc, VTH2, n2v[:, :, :, cur], AL.mult, AL.add)
        nc.vector.scalar_tensor_tensor(n2v[:, :, :, nxt], nvv, 0.5,
                                       i2v[:, :, :, s - 1], AL.mult, AL.subtract)
    off = 3 + (L2_S - 1 - (1 + L2_WU))
    nc.vector.tensor_scalar(sp2v[:, :, off:off + (L2_NC - 1) * L2_INT + 1:L2_INT],
                            n2v[:, :, :, (L2_S - 1) % 2], -VTH2, None, AL.is_lt)
    scrH.release()
    p_l2.release()
    if debug:
        dsp2 = dbg["dbg_sp2"][:].rearrange("p (b s) -> p b s", b=Bs)
        nc.sync.dma_start(dsp2[:, :, :], sp2v[:, :, 3:3 + T])

    # ============ phase H: conv2 per half + BN2 + build I3 ============
    p_l3 = P("p_l3", "left")
    I3a = p_l3.tile([C1, 2 * Bs * L3_NC * L3_I], F32, tag="I3a")
    I3b = p_l3.tile([C1, 2 * Bs * L3_NC * L3_I], F32, tag="I3b")
    nc.gpsimd.memset(I3a[:], 0.0)
    nc.gpsimd.memset(I3b[:], 0.0)
    i3av = I3a[:].rearrange("p (j c s) -> p j c s", j=2 * Bs, c=L3_NC)
    i3bv = I3b[:].rearrange("p (j c s) -> p j c s", j=2 * Bs, c=L3_NC)

    for h in range(2):
        p_c2 = P(f"p_c2_{h}", "right")
        c2_sb = p_c2.tile([C1, Bs * T], F32, tag="c2_sb")
        c2sum = p_c2.tile([C1, Bs], F32, tag="c2sum")
        c2sq = p_c2.tile([C1, Bs], F32, tag="c2sq")
        psH = tc.alloc_tile_pool(name=f"psH{h}", bufs=1, space="PSUM")
        scrI = P(f"scrI{h}", "right")
        for b in range(Bs):
            sl = slice(b * T, (b + 1) * T)
            c2_ps = psH.tile([C1, T], F32, tag="c2_ps", bufs=2)
            for k in range(7):
                nc.tensor.matmul(c2_ps[:], w2[:, (k * 2 + h) * C1:(k * 2 + h + 1) * C1],
                                 sp2v[:, b, k:k + T], start=(k == 0), stop=(k == 6))
            s2_ps = psH.tile([C1, T], F32, tag="s2_ps", bufs=2)
            nc.tensor.matmul(s2_ps[:], sc2w[:, h * C1:(h + 1) * C1],
                             sp2v[:, b, 3:3 + T], start=True, stop=True)
            nc.scalar.activation(c2_sb[:, sl], c2_ps[:], AF.Copy, accum_out=c2sum[:, b:b + 1])
            sqi = scrI.tile([C1, T], F32, tag="sqi", bufs=2)
            nc.scalar.activation(sqi[:], c2_ps[:], AF.Square, accum_out=c2sq[:, b:b + 1])
            j = h * Bs + b
            nc.scalar.activation(i3av[:, j, :, L3_WU:L3_I],
                                 s2_ps[:, 0:P1T].rearrange("p (c s) -> p c s", c=4), AF.Copy)
            nc.scalar.activation(i3bv[:, j, 0:3, L3_WU:L3_I],
                                 s2_ps[:, P1T:P1T + 192].rearrange("p (c s) -> p c s", c=3), AF.Copy)
            nc.scalar.activation(i3bv[:, j, 3, L3_WU:L3_WU + 32], s2_ps[:, P1T + 192:T], AF.Copy)
        scrI.release()
        psH.release()

        r0 = 16 + 16 * h
        nc.vector.tensor_reduce(stats[:, r0:r0 + 1], c2sum[:], axis=AX.X, op=AL.add)
        nc.vector.tensor_reduce(stats[:, r0 + 1:r0 + 2], c2sq[:], axis=AX.X, op=AL.add)
        ar3_i = pdram.tile([C1, 2], F32, tag=f"ar3_i{h}")
        ar3_o = pdram.tile([C1, 2], F32, tag=f"ar3_o{h}", addr_space="Shared")
        nc.sync.dma_start(ar3_i[:], stats[:, r0:r0 + 2])
        nc.gpsimd.collective_compute("AllReduce", AL.add, replica_groups=[core_ids],
                                     ins=[ar3_i.opt()], outs=[ar3_o.opt()])
        nc.sync.dma_start(stats[:, r0 + 2:r0 + 4], ar3_o[:])
        o0 = r0 + 2
        nc.vector.tensor_scalar(stats[:, o0 + 2:o0 + 3], stats[:, o0:o0 + 1], 1.0 / NBT, None, AL.mult)
        nc.vector.tensor_scalar(stats[:, o0 + 3:o0 + 4], stats[:, o0 + 1:o0 + 2], 1.0 / NBT, None, AL.mult)
        nc.vector.tensor_tensor(stats[:, o0 + 4:o0 + 5], stats[:, o0 + 2:o0 + 3], stats[:, o0 + 2:o0 + 3], AL.mult)
        nc.vector.tensor_tensor(stats[:, o0 + 4:o0 + 5], stats[:, o0 + 3:o0 + 4], stats[:, o0 + 4:o0 + 5], AL.subtract)
        nc.vector.tensor_scalar(stats[:, o0 + 4:o0 + 5], stats[:, o0 + 4:o0 + 5], 1e-5, None, AL.add)
        scrJ = P(f"scrJ{h}", "right")
        rstd2 = _rsqrt_refined(nc, scrJ, stats[:, o0 + 4:o0 + 5], C1, 1, f"bn2{h}")
        al2 = stats[:, o0 + 5:o0 + 6]
        nc.vector.tensor_scalar(al2, rstd2[:], g2[:, h:h + 1], None, AL.mult)
        nc.vector.tensor_scalar(stats[:, o0 + 6:o0 + 7], al2, -1.0, None, AL.mult)
        nc.vector.scalar_tensor_tensor(stats[:, o0 + 7:o0 + 8], stats[:, o0 + 6:o0 + 7],
                                       stats[:, o0 + 2:o0 + 3], b2s[:, h:h + 1], AL.mult, AL.add)
        c2bv = c2_sb[:].rearrange("p (b s) -> p b s", b=Bs)
        jsl = slice(h * Bs, (h + 1) * Bs)
        for c in range(L3_NC):
            tmp = scrJ.tile([C1, Bs * L3_INT], F32, tag="tmpj", bufs=2)
            tvj = tmp[:].rearrange("p (b s) -> p b s", b=Bs)
            nc.vector.tensor_scalar(tvj[:, :, :], c2bv[:, :, c * L3_INT:(c + 1) * L3_INT],
                                    al2, stats[:, o0 + 7:o0 + 8], AL.mult, AL.add)
            nc.vector.tensor_tensor(i3av[:, jsl, c, L3_WU:L3_I], tvj[:, :, :],
                                    i3av[:, jsl, c, L3_WU:L3_I], AL.add)
        for c in range(L3_NC):
            w = L3_INT if c < 3 else 32
            t0 = P1T + c * L3_INT
            tmp = scrJ.tile([C1, Bs * L3_INT], F32, tag="tmpj", bufs=2)
            tvj = tmp[:].rearrange("p (b s) -> p b s", b=Bs, s=L3_INT)
            nc.vector.tensor_scalar(tvj[:, :, 0:w], c2bv[:, :, t0:t0 + w],
                                    al2, stats[:, o0 + 7:o0 + 8], AL.mult, AL.add)
            nc.vector.tensor_tensor(i3bv[:, jsl, c, L3_WU:L3_WU + w], tvj[:, :, 0:w],
                                    i3bv[:, jsl, c, L3_WU:L3_WU + w], AL.add)
        scrJ.release()
        p_c2.release()
    p_sp2.release()

    # layer-3 warmup copies
    for c in range(1, L3_NC):
        nc.vector.tensor_scalar(i3av[:, :, c, 0:L3_WU],
                                i3av[:, :, c - 1, L3_I - L3_WU:L3_I], 1.0, None, AL.mult)
    nc.vector.tensor_scalar(i3bv[:, :, 0, 0:L3_WU],
                            i3av[:, :, 3, L3_I - L3_WU:L3_I], 1.0, None, AL.mult)
    for c in range(1, L3_NC):
        nc.vector.tensor_scalar(i3bv[:, :, c, 0:L3_WU],
                                i3bv[:, :, c - 1, L3_I - L3_WU:L3_I], 1.0, None, AL.mult)

    # ============ phase I: layer-3 scans + pooling ============
    p_sp3 = P("p_sp3", "right")
    sp3 = p_sp3.tile([C1, 2 * Bs * L3_NC * L3_INT], F32, tag="sp3")
    sp3v = sp3[:].rearrange("p (j c s) -> p j c s", j=2 * Bs, c=L3_NC)
    p_n3 = P("p_n3", "right")
    N3 = p_n3.tile([C1, 2 * Bs * L3_NC * 2], F32, tag="N3")
    n3v = N3[:].rearrange("p (j c s) -> p j c s", j=2 * Bs, c=L3_NC)

    for ps in range(2):
        i3v = i3av if ps == 0 else i3bv
        nc.gpsimd.memset(n3v[:, :, :, 0], 0.0)
        scrK = P(f"scrK{ps}", "right")
        for s in range(1, L3_S):
            cur, nxt = (s - 1) % 2, s % 2
            if s - 1 >= 1 + L3_WU:
                sp_loc = sp3v[:, :, :, s - 1 - (1 + L3_WU)]
            else:
                spscr = scrK.tile([C1, 2 * Bs * L3_NC], F32, tag="spscr3", bufs=2)
                sp_loc = spscr[:].rearrange("p (j c) -> p j c", j=2 * Bs)
            nc.vector.tensor_scalar(sp_loc, n3v[:, :, :, cur], -VTH2, None, AL.is_lt)
            nv = scrK.tile([C1, 2 * Bs * L3_NC], F32, tag="nv3", bufs=2)
            nvv = nv[:].rearrange("p (j c) -> p j c", j=2 * Bs)
            nc.vector.scalar_tensor_tensor(nvv, sp_loc, VTH2, n3v[:, :, :, cur], AL.mult, AL.add)
            nc.vector.scalar_tensor_tensor(n3v[:, :, :, nxt], nvv, 0.5,
                                           i3v[:, :, :, s - 1], AL.mult, AL.subtract)
        nc.vector.tensor_scalar(sp3v[:, :, :, L3_INT - 1], n3v[:, :, :, (L3_S - 1) % 2],
                                -VTH2, None, AL.is_lt)
        scrK.release()
        if debug:
            d3 = dbg["dbg_sp3"][:].rearrange("p (q j c s) -> p q j c s", q=2, j=2 * Bs, c=L3_NC)
            for c in range(L3_NC):
                nc.sync.dma_start(d3[:, ps, :, c, :], sp3v[:, :, c, :])
        # pooling: 32-wide windows, 2 per 64-wide chunk
        for h in range(2):
            jsl = slice(h * Bs, (h + 1) * Bs)
            for hw in range(2):
                nch = 3 if (ps == 1 and hw == 1) else L3_NC
                csl = slice(0, nch)
                wbase = ps * 8 + hw
                xv = X[:].rearrange("p (q r) -> p q r", r=17)
                st = h * 15 + wbase
                inv = sp3v[:, jsl, csl, hw * 32:(hw + 1) * 32]
                outv = xv[:, st:st + 2 * (nch - 1) + 1:2, 0:Bs].rearrange("p w b -> p b w")
                nc.vector.tensor_reduce(outv, inv, axis=AX.X, op=AL.add)
    p_n3.release()
    p_sp3.release()
    p_l3.release()

    # ============ phase J: prefc BN + FC ============
    xv3 = X[:].rearrange("p (q r) -> p q r", r=17)
    scrL = P("scrL", "left")
    xsq = scrL.tile([C1, 30 * 16], F32, tag="xsq")
    xsqv = xsq[:].rearrange("p (q r) -> p q r", r=16)
    nc.scalar.activation(xsqv[:, :, :], xv3[:, :, 0:Bs], AF.Square)
    st4 = scrL.tile([C1, 4 * 30], F32, tag="st4")
    nc.vector.tensor_reduce(st4[:, 0:30], xv3[:, :, 0:Bs], axis=AX.X, op=AL.add)
    nc.vector.tensor_reduce(st4[:, 30:60], xsqv[:, :, :], axis=AX.X, op=AL.add)
    ar4_i = pdram.tile([C1, 60], F32, tag="ar4_i")
    ar4_o = pdram.tile([C1, 60], F32, tag="ar4_o", addr_space="Shared")
    nc.sync.dma_start(ar4_i[:], st4[:, 0:60])
    nc.gpsimd.collective_compute("AllReduce", AL.add, replica_groups=[core_ids],
                                 ins=[ar4_i.opt()], outs=[ar4_o.opt()])
    nc.sync.dma_start(st4[:, 60:120], ar4_o[:])
    m4 = scrL.tile([C1, 30], F32, tag="m4")
    nc.vector.tensor_scalar(m4[:], st4[:, 60:90], 1.0 / 128.0, None, AL.mult)
    e4 = scrL.tile([C1, 30], F32, tag="e4")
    nc.vector.tensor_scalar(e4[:], st4[:, 90:120], 1.0 / 128.0, None, AL.mult)
    v4 = scrL.tile([C1, 30], F32, tag="v4")
    nc.vector.tensor_tensor(v4[:], m4[:], m4[:], AL.mult)
    nc.vector.tensor_tensor(v4[:], e4[:], v4[:], AL.subtract)
    nc.vector.tensor_scalar(v4[:], v4[:], 1.0 / 1024.0, 1e-5, AL.mult, AL.add)
    rstd4 = _rsqrt_refined(nc, scrL, v4[:], C1, 30, "bnfc")
    G = scrL.tile([C1, 30 * 4], F32, tag="G")
    gv = G[:].rearrange("p (q r) -> p q r", r=4)
    gfcv = gfc[:].rearrange("p (q r) -> p q r", r=4)
    for cch in range(30):
        nc.vector.tensor_scalar(gv[:, cch, :], gfcv[:, cch, :],
                                rstd4[:, cch:cch + 1], None, AL.mult)
    nc.vector.tensor_scalar(xv3[:, :, 16], m4[:], 1.0, None, AL.mult)
    if debug:
        nc.sync.dma_start(dbg["dbg_x"][:], X[:])

    psJ = tc.alloc_tile_pool(name="psJ", bufs=1, space="PSUM")
    fc_ps = psJ.tile([4, 17], F32, tag="fc_ps")
    for cch in range(30):
        nc.tensor.matmul(fc_ps[:], gv[:, cch, :], xv3[:, cch, :],
                         start=(cch == 0), stop=(cch == 29))
    mcol = scrL.tile([4, 1], F32, tag="mcol")
    nc.scalar.activation(mcol[:], fc_ps[:, 16:17], AF.Copy)
    ofin = scrL.tile([4, Bs], F32, tag="ofin")
    nc.vector.tensor_scalar(ofin[:], fc_ps[:, 0:16], mcol[:], None, AL.subtract)
    nc.vector.tensor_scalar(ofin[:], ofin[:], hfc[:], None, AL.add)
    nc.sync.dma_start(o_out[:], ofin[:])
    psJ.release()
    scrL.release()

    p0.release()
    pdram.release()


# ======================= host side =======================

def _host_prep(inputs):
    f64 = np.float64
    f32 = np.float32
    feats = np.asarray(inputs['features'])
    A = np.asarray(inputs['A_norm']); Wu = np.asarray(inputs['Wu_w']); Wv = np.asarray(inputs['Wv_w'])
    conv1_w = np.asarray(inputs['conv1_w']); sc1_w = np.asarray(inputs['sc1_w'])
    conv2_w = np.asarray(inputs['conv2_w']); sc2_w = np.asarray(inputs['sc2_w'])

    u = feats[..., 0]; v = feats[..., 1]; curv = feats[..., 2]; tang = feats[..., 3]
    e = np.exp(-(f32(0.8) * curv + f32(0.4) * tang), dtype=f32)
    tau = (f32(35.0) * e).astype(f32)
    dec = np.exp(f32(-1.0) / tau, dtype=f32)

    w1p = np.zeros((C1, 8 * C1), f32)
    for j in range(8):
        w1p[0:C, j * C1:(j + 1) * C1] = conv1_w[:, :, 2 * j].T
        if 2 * j + 1 < 15:
            w1p[C:C1, j * C1:(j + 1) * C1] = conv1_w[:, :, 2 * j + 1].T
    w2 = np.zeros((C1, 14 * C1), f32)
    for k in range(7):
        for h in range(2):
            w2[:, (k * 2 + h) * C1:(k * 2 + h + 1) * C1] = conv2_w[h * C1:(h + 1) * C1, :, k].T
    sc2 = np.zeros((C1, 2 * C1), f32)
    for h in range(2):
        sc2[:, h * C1:(h + 1) * C1] = sc2_w[h * C1:(h + 1) * C1, :, 0].T

    gp = np.asarray(inputs['prefc_g']).astype(f64)
    bp = np.asarray(inputs['prefc_b']).astype(f64)
    fcw = np.asarray(inputs['fc_w']).astype(f64)
    gfc = np.zeros((C1, 30 * 4), f32)
    for half in range(2):
        for w in range(15):
            cch = half * 15 + w
            fidx = (half * C1 + np.arange(C1)) * 15 + w
            gfc[:, cch * 4:(cch + 1) * 4] = (fcw[:, fidx] * gp[fidx] / 32.0).T.astype(f32)
    hfc = (np.asarray(inputs['fc_b']).astype(f64) + fcw @ bp).astype(f32).reshape(4, 1)

    shared = {
        "a_w": np.ascontiguousarray(A.astype(f32)),
        "wu_w": np.ascontiguousarray(Wu.T.astype(f32)),
        "wv_w": np.ascontiguousarray(Wv.T.astype(f32)),
        "ginj": np.asarray(inputs['bn_inj_g']).astype(f32).reshape(C, 1),
        "binj": np.asarray(inputs['bn_inj_b']).astype(f32).reshape(C, 1),
        "w1p": w1p,
        "sc1w": np.ascontiguousarray(sc1_w[:, :, 0].T.astype(f32)),
        "g1": np.asarray(inputs['bn1_g']).astype(f32).reshape(C1, 1),
        "b1s": (np.asarray(inputs['bn1_b']).astype(f64)
                + np.asarray(inputs['sc1_b']).astype(f64)).astype(f32).reshape(C1, 1),
        "w2": w2,
        "sc2w": sc2,
        "g2": np.ascontiguousarray(np.asarray(inputs['bn2_g']).astype(f32).reshape(2, C1).T),
        "b2s": np.ascontiguousarray(
            (np.asarray(inputs['bn2_b']).astype(f64)
             + np.asarray(inputs['sc2_b']).astype(f64)).astype(f32).reshape(2, C1).T),
        "gfc": gfc,
        "hfc": hfc,
    }
    in_maps = []
    for k in range(NCORES):
        bs = slice(k * Bs, (k + 1) * Bs)
        m = dict(shared)
        m["u"] = np.ascontiguousarray(u[bs].transpose(1, 0, 2).reshape(C, Bs * T))
        m["v"] = np.ascontiguousarray(v[bs].transpose(1, 0, 2).reshape(C, Bs * T))
        m["dec"] = np.ascontiguousarray(dec[bs].transpose(1, 0, 2).reshape(C, Bs * T))
        in_maps.append(m)
    return in_maps


_NC_CACHE = {}


def _get_nc(debug=False, repeat=1):
    key = (debug, repeat)
    if key not in _NC_CACHE:
        _NC_CACHE[key] = build(debug=debug, repeat=repeat)
    return _NC_CACHE[key]


def run(inputs, debug=False, repeat=1):
    in_maps = _host_prep(inputs)
    nc = _get_nc(debug=debug, repeat=repeat)
    res = run_bass_kernel_spmd(nc, in_maps, list(range(NCORES)))
    out = np.concatenate([res.results[k]["o"].T for k in range(NCORES)], axis=0)
    return out.astype(np.float32), res


def kernel(**inputs) -> np.ndarray:
    out, _ = run(inputs)
    return out
